# revision 1
# baseline (speedup 1.0000x reference)
"""MoE layer (E=8 experts, top-2, swiGLU) on 8 TRN2 NeuronCores.

Strategy: expert parallelism. The router (x @ Wr -> top-2 -> softmax gates)
is tiny (<0.1% of FLOPs) and is computed on host to build the dispatch:
tokens are gathered per expert into a padded capacity-C batch, one expert
per core. Each core runs the expert MLP

    y = (silu(X @ W1a + b1a) * (X @ W1b + b1b)) @ W2 + b2, scaled by gate

entirely on device with float32r matmuls (full-rate PE, ~fp32 accuracy).
The host scatter-adds the 8 per-expert outputs back (each token appears in
exactly 2 expert batches) — the EP "combine" step.

Device kernel structure (per core, SPMD — identical program, per-core data):
  - xt  [P, KO1, C]   token block, transposed (D on partitions)   (resident)
  - w2  [P, KO2, D]   expert W2                                    (resident)
  - hT  [P, MP, TB]   swiGLU output, transposed (H on partitions) (per block)
  - W1 streamed from HBM in [P, 2, KO1, 128] column tiles
  GEMM1: H1T[h, t] = sum_k W1[k, h] * X[t, k]   (stationary=W1 tile, moving=xt)
  GEMM2: Y[t, d]   = sum_h hT[h, t] * W2[h, d]  (stationary=hT tile, moving=w2)
  Token dim processed in blocks of TB=512 so hT fits in SBUF; W1 is
  re-streamed once per block.
"""

import math

import numpy as np

import concourse.bacc as bacc
import concourse.bass as bass  # noqa: F401
import concourse.mybir as mybir
import concourse.tile as tile
from concourse.bass_utils import run_bass_kernel_spmd
from concourse.tile import add_dep_helper

P = 128
NCORES = 8

f32 = mybir.dt.float32
f32r = mybir.dt.float32r
SIGMOID = mybir.ActivationFunctionType.Sigmoid
ADD = mybir.AluOpType.add


def _blocks(C, TB):
    """Split C (multiple of 128) into token blocks of at most TB.
    Any remainder block goes FIRST: the small block's swiGLU-latency
    stalls then overlap the DMA-warmup phase instead of the tail."""
    rem = C % TB
    out = []
    t0 = 0
    if rem:
        out.append((0, rem))
        t0 = rem
    while t0 < C:
        out.append((t0, TB))
        t0 += TB
    return out


def _chunks(tw):
    """Split a token block into matmul moving-dim chunks (<=512)."""
    out = []
    c0 = 0
    while c0 < tw:
        out.append((c0, min(512, tw - c0)))
        c0 += 512
    return out


def build_moe_expert_nc(D, H, C, TB=768, has_b1=False, has_b2=False):
    """Build the SPMD per-expert kernel. D % 128 == 0, H % 128 == 0,
    C % 128 == 0 required."""
    KO1 = D // P       # k tiles of GEMM1 (contraction over D)
    MP = H // P        # hidden tiles (per swiGLU half)
    KO2 = H // P       # k tiles of GEMM2 (contraction over H)
    n2chunks = _chunks(D)  # GEMM2 free-dim chunks over D

    nc = bacc.Bacc(None)
    xt_d = nc.declare_dram_parameter("xt", [P, KO1, C], f32r, isOutput=False)
    w1_d = nc.declare_dram_parameter("w1", [MP, P, 2, KO1, P], f32r, isOutput=False)
    w2_d = nc.declare_dram_parameter("w2", [P, KO2, D], f32r, isOutput=False)
    g_d = nc.declare_dram_parameter("g", [P, C // P], f32, isOutput=False)
    if has_b1:
        b1_d = nc.declare_dram_parameter("b1", [P, 2, MP], f32, isOutput=False)
    if has_b2:
        b2_d = nc.declare_dram_parameter("b2", [P, D], f32, isOutput=False)
    y_d = nc.declare_dram_parameter("y", [C, D], f32, isOutput=True)

    blocks = _blocks(C, TB)

    with tile.TileContext(nc) as tc:
        with (
            tc.tile_pool(name="const", bufs=1) as const,
            tc.tile_pool(name="xtp", bufs=2) as xtp,
            tc.tile_pool(name="w1p", bufs=3) as w1p,
            tc.tile_pool(name="ev", bufs=2) as ev,
            tc.tile_pool(name="ps1", bufs=1, space="PSUM") as ps1,
            tc.tile_pool(name="ps2", bufs=2, space="PSUM") as ps2,
        ):
            # Only block 0's tokens load on the sync (HWDGE) queues before
            # compute: HWDGE waits are cumulative per queue, so anything
            # enqueued ahead of the first W1 tile would stall the first
            # matmul. The remaining bulk loads (xt blocks 1+, w2, gates,
            # biases) go on gpsimd (SWDGE) queues with explicit dep edges
            # so they stream DURING compute instead of starving the W1
            # prefetch at t=0.
            # xt blocks share one double-buffered tag: block i+1's tokens
            # stream in (during block i's GEMM1) over the buffer freed when
            # block i-1's GEMM1 retired.
            xt_tiles = [
                xtp.tile([P, KO1, tw], f32r, name=f"xt{bi}", tag="xt")
                if not (bi == 0 and tw > 512)
                else None
                for bi, (t0, tw) in enumerate(blocks)
            ]
            g_sb = const.tile([P, C // P], f32)
            w2_sb = const.tile([P, KO2, D], f32r)
            if has_b1:
                # tiny; read by the very first swiGLU, so load up front
                b1_sb = const.tile([P, 2, MP], f32)
                nc.sync.dma_start(b1_sb[:], b1_d[:])
            if has_b2:
                b2_sb = const.tile([P, D], f32)
                nc.sync.dma_start(b2_sb[:], b2_d[:])
            tw0 = blocks[0][1]
            if xt_tiles[0] is None:
                # split block 0: only chunk 0 (cols 0:512) gates the first
                # matmul on the sync queue; the rest rides SWDGE in parallel
                # and is not needed until the second chunk's matmuls.
                xt0a = xtp.tile([P, KO1, 512], f32r, tag="xt")
                xt0b = xtp.tile([P, KO1, tw0 - 512], f32r, tag="xt")
                nc.sync.dma_start(xt0a[:], xt_d[:, :, :512])
                nc.gpsimd.dma_start(xt0b[:], xt_d[:, :, 512:tw0])
            else:
                xt0a, xt0b = xt_tiles[0], None
                nc.sync.dma_start(xt0a[:], xt_d[:, :, :tw0])

            def xt_rhs(bi, k, c0, cw):
                if bi == 0:
                    if c0 < 512:
                        return xt0a[:, k, c0 : c0 + cw]
                    return xt0b[:, k, c0 - 512 : c0 - 512 + cw]
                return xt_tiles[bi][:, k, c0 : c0 + cw]

            # PE warmup: ~4us of throwaway matmuls on a zeroed scratch tile
            # while the first real loads stream in, so the HAM clock gate is
            # already at 2.4 GHz when real matmuls start.
            warm = const.tile([P, 640], f32r)
            nc.gpsimd.memset(warm[:].bitcast(f32), 0.0)
            for wi in range(13):
                warm_ps = ps1.tile([P, 512], f32, tag="g1_2", name=f"warm_ps{wi}")
                nc.tensor.matmul(
                    warm_ps[:],
                    lhsT=warm[:, :128],
                    rhs=warm[:, 128:640],
                    start=True,
                    stop=True,
                )

            # filled during the main loop: first matmul of (block, mp)
            block_mm = {}

            def _stagger(dma_bi, anchor):
                if anchor is not None:
                    add_dep_helper(
                        dma_bi.ins, anchor.ins, sync=True,
                        reason="stagger bulk DMA behind compute",
                    )

            for bi, (t0, tw) in enumerate(blocks):
                hT = ev.tile([P, MP, tw], f32r, tag="hT", bufs=1)
                # ---- GEMM1 + swiGLU: hT[:, mp, :] for all hidden tiles ----
                for mp in range(MP):
                    w1t = w1p.tile([P, 2, KO1, P], f32r, tag="w1t")
                    nc.sync.dma_start(w1t[:], w1_d[mp])
                    # 3-tag PSUM rotation: reuse distance 1.5 mp-pairs, so
                    # the ACT->DVE->DVE swiGLU chain latency never stalls
                    # the next mp's matmul group.
                    psa = ps1.tile([P, tw], f32, tag=f"g1_{(2 * mp) % 3}")
                    psb = ps1.tile([P, tw], f32, tag=f"g1_{(2 * mp + 1) % 3}")
                    for c0, cw in _chunks(tw):
                        for k in range(KO1):
                            mm = nc.tensor.matmul(
                                psa[:, c0 : c0 + cw],
                                lhsT=w1t[:, 0, k, :],
                                rhs=xt_rhs(bi, k, c0, cw),
                                start=(k == 0),
                                stop=(k == KO1 - 1),
                            )
                            block_mm.setdefault((bi, mp), mm)
                        for k in range(KO1):
                            nc.tensor.matmul(
                                psb[:, c0 : c0 + cw],
                                lhsT=w1t[:, 1, k, :],
                                rhs=xt_rhs(bi, k, c0, cw),
                                start=(k == 0),
                                stop=(k == KO1 - 1),
                            )
                    # silu(a) = a * sigmoid(a); a = psa (+ b1a), b = psb (+ b1b)
                    sg = ev.tile([P, tw], f32, tag="sg")
                    if has_b1:
                        av = ev.tile([P, tw], f32, tag="av")
                        nc.vector.tensor_scalar_add(
                            av[:], psa[:], b1_sb[:, 0, mp : mp + 1]
                        )
                        nc.scalar.activation(sg[:], av[:], SIGMOID)
                        nc.vector.tensor_mul(sg[:], sg[:], av[:])
                        bs = ev.tile([P, tw], f32, tag="bs")
                        nc.vector.tensor_scalar_add(
                            bs[:], psb[:], b1_sb[:, 1, mp : mp + 1]
                        )
                        nc.vector.tensor_mul(hT[:, mp, :], sg[:], bs[:])
                    else:
                        nc.scalar.activation(sg[:], psa[:], SIGMOID)
                        nc.vector.tensor_mul(sg[:], sg[:], psa[:])
                        nc.vector.tensor_mul(hT[:, mp, :], sg[:], psb[:])

                # ---- staggered bulk loads: emitted BEFORE their readers
                # (Tile deps are emission-ordered) but dep-anchored on this
                # block's first matmul so they stream during compute instead
                # of starving the W1/xt0 critical path at t=0.
                if bi == 0:
                    # w2 in 4 chunks spread across block 0's GEMM1 so the
                    # SWDGE bursts never starve the W1 stream for long
                    nw2 = 4
                    kstep = max(1, KO2 // nw2)
                    for ci, k0 in enumerate(range(0, KO2, kstep)):
                        k1 = min(KO2, k0 + kstep)
                        dma = nc.gpsimd.dma_start(
                            w2_sb[:, k0:k1, :], w2_d[:, k0:k1, :]
                        )
                        anchor_mp = min(5 + 3 * ci, MP - 1)
                        _stagger(dma, block_mm.get((0, anchor_mp)))
                    dma = nc.gpsimd.dma_start(g_sb[:], g_d[:])
                    _stagger(dma, block_mm.get((0, 0)))
                if bi + 1 < len(blocks):
                    # next block's tokens stream during THIS block's GEMM2
                    # (a window with no W1 demand)
                    u0, uw = blocks[bi + 1]
                    dma = nc.gpsimd.dma_start(
                        xt_tiles[bi + 1][:], xt_d[:, :, u0 : u0 + uw]
                    )
                    _stagger(dma, block_mm.get((bi, MP - 1)))

                # ---- GEMM2 + gate scale: y rows for this token block ----
                for mt in range(tw // P):
                    ti = t0 // P + mt
                    rows = slice(t0 + mt * P, t0 + (mt + 1) * P)
                    for n0, nw in n2chunks:
                        psy = ps2.tile([P, max(nw, 1)], f32, tag="psy")
                        for k in range(KO2):
                            nc.tensor.matmul(
                                psy[:, :nw],
                                lhsT=hT[:, k, mt * P : (mt + 1) * P],
                                rhs=w2_sb[:, k, n0 : n0 + nw],
                                start=(k == 0),
                                stop=(k == KO2 - 1),
                            )
                        ysb = ev.tile([P, nw], f32, tag="ysb")
                        if has_b2:
                            nc.vector.tensor_tensor(
                                ysb[:], psy[:, :nw], b2_sb[:, n0 : n0 + nw], ADD
                            )
                            nc.vector.tensor_scalar_mul(
                                ysb[:], ysb[:], g_sb[:, ti : ti + 1]
                            )
                        else:
                            nc.vector.tensor_scalar_mul(
                                ysb[:], psy[:, :nw], g_sb[:, ti : ti + 1]
                            )
                        nc.sync.dma_start(y_d[rows, n0 : n0 + nw], ysb[:])
    # run_bass_via_pjrt (the axon execute path) takes a prebuilt module and
    # never finalizes it; Bacc defers register allocation to finalize().
    nc.finalize()
    return nc


def _route(x2, Wr):
    """Top-2 router, numpy fp32 (mirrors jax.lax.top_k + softmax)."""
    n = x2.shape[0]
    ar = np.arange(n)
    z = x2 @ Wr  # [N, E] fp32
    idx1 = z.argmax(axis=1)
    v1 = z[ar, idx1]
    z2 = z.copy()
    z2[ar, idx1] = -np.inf
    idx2 = z2.argmax(axis=1)
    v2 = z2[ar, idx2]
    m = np.maximum(v1, v2)
    e1 = np.exp(v1 - m)
    e2 = np.exp(v2 - m)
    s = e1 + e2
    return idx1, idx2, (e1 / s).astype(np.float32), (e2 / s).astype(np.float32)


def kernel(x, Wr, W1, b1, W2, b2):
    x = np.asarray(x, dtype=np.float32)
    Wr = np.asarray(Wr, dtype=np.float32)
    W1 = np.asarray(W1, dtype=np.float32)
    b1 = np.asarray(b1, dtype=np.float32)
    W2 = np.asarray(W2, dtype=np.float32)
    b2 = np.asarray(b2, dtype=np.float32)

    Bb, T, D = x.shape
    E, _, H2 = W1.shape
    H = H2 // 2
    N = Bb * T
    assert E == NCORES

    x2 = x.reshape(N, D)
    idx1, idx2, g1, g2 = _route(x2, Wr)

    tok = np.concatenate([np.arange(N), np.arange(N)])
    exp = np.concatenate([idx1, idx2])
    gat = np.concatenate([g1, g2])

    toks_e = [tok[exp == e] for e in range(E)]
    gats_e = [gat[exp == e] for e in range(E)]
    counts = np.array([len(t) for t in toks_e])
    C = max(512, int(math.ceil(counts.max() / P) * P))

    has_b1 = bool(np.any(b1))
    has_b2 = bool(np.any(b2))

    nc = build_moe_expert_nc(D, H, C, TB=768, has_b1=has_b1, has_b2=has_b2)

    KO1 = D // P
    MP = H // P
    KO2 = H // P

    in_maps = []
    for e in range(E):
        ce = len(toks_e[e])
        xt = np.zeros((D, C), dtype=np.float32)
        xt[:, :ce] = x2[toks_e[e]].T
        xt_t = np.ascontiguousarray(xt.reshape(KO1, P, C).transpose(1, 0, 2))

        w1_t = np.ascontiguousarray(
            W1[e].reshape(KO1, P, 2, MP, P).transpose(3, 1, 2, 0, 4)
        )
        w2_t = np.ascontiguousarray(W2[e].reshape(KO2, P, D).transpose(1, 0, 2))

        g = np.zeros(C, dtype=np.float32)
        g[:ce] = gats_e[e]
        g_t = np.ascontiguousarray(g.reshape(C // P, P).T)

        im = {"xt": xt_t, "w1": w1_t, "w2": w2_t, "g": g_t}
        if has_b1:
            im["b1"] = np.ascontiguousarray(
                b1[e].reshape(2, MP, P).transpose(2, 0, 1)
            )
        if has_b2:
            im["b2"] = np.ascontiguousarray(np.broadcast_to(b2[e], (P, D)))
        in_maps.append(im)

    res = run_bass_kernel_spmd(nc, in_maps, list(range(NCORES)))

    out = np.zeros((N, D), dtype=np.float32)
    for e in range(E):
        ce = len(toks_e[e])
        out[toks_e[e]] += res.results[e]["y"][:ce]
    return out.reshape(Bb, T, D)



# revision 2
# speedup vs baseline: 1.1313x; 1.1313x over previous
"""MoE layer (E=8 experts, top-2, swiGLU) on 8 TRN2 NeuronCores.

Strategy: expert parallelism. The router (x @ Wr -> top-2 -> softmax gates)
is tiny (<0.1% of FLOPs) and is computed on host to build the dispatch:
tokens are gathered per expert into a padded capacity-C batch, one expert
per core. Each core runs the expert MLP

    y = (silu(X @ W1a + b1a) * (X @ W1b + b1b)) @ W2 + b2, scaled by gate

entirely on device in bf16 (err ~4e-3 << 2e-2 gate; bf16 matmul streams at
the same 1 col/cycle as f32r but halves DMA traffic and enables FWL fast
weight loads). The host scatter-adds the 8 per-expert outputs back (each
token appears in exactly 2 expert batches) — the EP "combine" step.

Device kernel structure (per core, SPMD — identical program, per-core data):
  Single token block (no TB loop): W1 streamed exactly once.
  - xt  [P, KO1*C]   tokens, transposed, chunk-major packed      (resident)
  - w2  [P, KO2, D]  expert W2                                   (resident)
  - hT  [P, MP, C]   swiGLU output, transposed (H on partitions) (resident)
  - W1 streamed from HBM in [P, 2, KO1, 128] column tiles, one per mp
  GEMM1: H1T[h, t] = sum_k W1[k, h] * X[t, k]   (stationary=W1, moving=xt)
  GEMM2: Y[t, d]   = sum_h hT[h, t] * W2[h, d]  (stationary=hT, moving=w2)
  Token dim processed in moving chunks of <=512 (PSUM bank limit); the
  C%512 remainder chunk goes FIRST so the prologue's first matmul group
  only needs a small xt transfer.
  Sync (HWDGE) queue order = critical path order: w1t[0] half-a, xt chunk
  0, w1t[0] half-b, xt chunks 1.., then w1t[1..15], then y writes. Bulk
  w2/g ride SWDGE (gpsimd) queues dep-anchored behind early matmuls.
"""

import math

import numpy as np
import ml_dtypes

import concourse.bacc as bacc
import concourse.bass as bass  # noqa: F401
import concourse.mybir as mybir
import concourse.tile as tile
from concourse.bass_utils import run_bass_kernel_spmd
from concourse.tile import add_dep_helper

P = 128
NCORES = 8

f32 = mybir.dt.float32
f32r = mybir.dt.float32r
bf16 = mybir.dt.bfloat16
SIGMOID = mybir.ActivationFunctionType.Sigmoid
ADD = mybir.AluOpType.add

NP_BF16 = ml_dtypes.bfloat16


def _chunks(C):
    """Moving-dim chunks of <=512 covering C, remainder chunk FIRST (it
    gates the prologue: the first matmul group then only needs the small
    chunk in SBUF)."""
    rem = C % 512
    out = []
    c0 = 0
    if rem:
        out.append((0, rem))
        c0 = rem
    while c0 < C:
        out.append((c0, 512))
        c0 += 512
    return out


def build_moe_expert_nc(D, H, C, has_b1=False, has_b2=False):
    """Build the SPMD per-expert kernel. D % 128 == 0, H % 128 == 0,
    C % 16 == 0 required."""
    KO1 = D // P       # k tiles of GEMM1 (contraction over D)
    MP = H // P        # hidden tiles (per swiGLU half)
    KO2 = H // P       # k tiles of GEMM2 (contraction over H)
    TT = (C + P - 1) // P   # GEMM2 token tiles (last may be partial)
    chunks = _chunks(C)
    # flat free-dim offset of each chunk in the chunk-major packed xt
    xt_off = [KO1 * c0 for c0, _ in chunks]

    nc = bacc.Bacc(None)
    xt_d = nc.declare_dram_parameter("xt", [P, KO1 * C], bf16, isOutput=False)
    w1_d = nc.declare_dram_parameter("w1", [MP, P, 2, KO1, P], bf16, isOutput=False)
    w2_d = nc.declare_dram_parameter("w2", [P, KO2, D], bf16, isOutput=False)
    g_d = nc.declare_dram_parameter("g", [P, TT], f32, isOutput=False)
    if has_b1:
        b1_d = nc.declare_dram_parameter("b1", [P, 2, MP], f32, isOutput=False)
    if has_b2:
        b2_d = nc.declare_dram_parameter("b2", [P, D], f32, isOutput=False)
    y_d = nc.declare_dram_parameter("y", [C, D], f32, isOutput=True)

    with tile.TileContext(nc) as tc:
        with (
            tc.tile_pool(name="const", bufs=1) as const,
            tc.tile_pool(name="w1p", bufs=3) as w1p,
            tc.tile_pool(name="ev", bufs=2) as ev,
            tc.tile_pool(name="ps1", bufs=1, space="PSUM") as ps1,
            tc.tile_pool(name="ps2", bufs=1, space="PSUM") as ps2,
        ):
            xt_sb = const.tile([P, KO1 * C], bf16)
            g_sb = const.tile([P, TT], f32)
            w2_sb = const.tile([P, KO2, D], bf16)
            hT = const.tile([P, MP, C], bf16)
            if has_b1:
                b1_sb = const.tile([P, 2, MP], f32)
                nc.sync.dma_start(b1_sb[:], b1_d[:])
            if has_b2:
                b2_sb = const.tile([P, D], f32)
                nc.sync.dma_start(b2_sb[:], b2_d[:])

            # PE warmup: throwaway matmuls on a zeroed scratch tile while the
            # first real loads stream in, so the HAM clock gate is already at
            # 2.4 GHz when real matmuls start. DVE memset (fast) so warmups
            # begin almost immediately.
            warm = const.tile([P, 640], f32r)
            nc.vector.memset(warm[:].bitcast(f32), 0.0)
            for wi in range(13):
                warm_ps = ps1.tile([P, 512], f32, tag=f"g1_{wi % 6}",
                                   name=f"warm_ps{wi}")
                nc.tensor.matmul(
                    warm_ps[:],
                    lhsT=warm[:, :128],
                    rhs=warm[:, 128:640],
                    start=True,
                    stop=True,
                )

            # ---- critical-path sync-queue loads (HWDGE waits are cumulative
            # per queue, so emission order here IS arrival order) ----
            w1t0 = w1p.tile([P, 2, KO1, P], bf16, tag="w1t", name="w1t0")
            c00, cw0 = chunks[0]
            nc.sync.dma_start(w1t0[:, 0], w1_d[0, :, 0])          # half a
            nc.sync.dma_start(
                xt_sb[:, xt_off[0] : xt_off[0] + KO1 * cw0],
                xt_d[:, xt_off[0] : xt_off[0] + KO1 * cw0],
            )                                                      # chunk 0
            nc.sync.dma_start(w1t0[:, 1], w1_d[0, :, 1])          # half b
            for ci in range(1, len(chunks)):
                c0, cw = chunks[ci]
                nc.sync.dma_start(
                    xt_sb[:, xt_off[ci] : xt_off[ci] + KO1 * cw],
                    xt_d[:, xt_off[ci] : xt_off[ci] + KO1 * cw],
                )

            def xt_rhs(ci, k, cw):
                off = xt_off[ci] + k * cw
                return xt_sb[:, off : off + cw]

            first_mm = {}   # mp -> first matmul (dep anchor for bulk DMA)

            def _stagger(dma_bi, anchor):
                if anchor is not None:
                    add_dep_helper(
                        dma_bi.ins, anchor.ins, sync=True,
                        reason="stagger bulk DMA behind compute",
                    )

            # ---- GEMM1 + swiGLU ----
            ic = 0
            for mp in range(MP):
                if mp == 0:
                    w1t = w1t0
                else:
                    w1t = w1p.tile([P, 2, KO1, P], bf16, tag="w1t")
                    nc.sync.dma_start(w1t[:], w1_d[mp])
                for ci, (c0, cw) in enumerate(chunks):
                    psa = ps1.tile([P, 512], f32, tag=f"g1_{(2 * ic) % 6}")
                    psb = ps1.tile([P, 512], f32, tag=f"g1_{(2 * ic + 1) % 6}")
                    ic += 1
                    for k in range(KO1):
                        mm = nc.tensor.matmul(
                            psa[:, :cw],
                            lhsT=w1t[:, 0, k, :],
                            rhs=xt_rhs(ci, k, cw),
                            start=(k == 0),
                            stop=(k == KO1 - 1),
                        )
                        first_mm.setdefault(mp, mm)
                    for k in range(KO1):
                        nc.tensor.matmul(
                            psb[:, :cw],
                            lhsT=w1t[:, 1, k, :],
                            rhs=xt_rhs(ci, k, cw),
                            start=(k == 0),
                            stop=(k == KO1 - 1),
                        )
                    # silu(a) = a * sigmoid(a); a = psa (+ b1a), b = psb (+ b1b)
                    sg = ev.tile([P, 512], f32, tag="sg")
                    if has_b1:
                        av = ev.tile([P, 512], f32, tag="av")
                        nc.vector.tensor_scalar_add(
                            av[:, :cw], psa[:, :cw], b1_sb[:, 0, mp : mp + 1]
                        )
                        nc.scalar.activation(sg[:, :cw], av[:, :cw], SIGMOID)
                        nc.vector.tensor_mul(sg[:, :cw], sg[:, :cw], av[:, :cw])
                        bs = ev.tile([P, 512], f32, tag="bs")
                        nc.vector.tensor_scalar_add(
                            bs[:, :cw], psb[:, :cw], b1_sb[:, 1, mp : mp + 1]
                        )
                        nc.vector.tensor_mul(
                            hT[:, mp, c0 : c0 + cw], sg[:, :cw], bs[:, :cw]
                        )
                    else:
                        nc.scalar.activation(sg[:, :cw], psa[:, :cw], SIGMOID)
                        nc.vector.tensor_mul(sg[:, :cw], sg[:, :cw], psa[:, :cw])
                        nc.vector.tensor_mul(
                            hT[:, mp, c0 : c0 + cw], sg[:, :cw], psb[:, :cw]
                        )

                # ---- staggered bulk loads: emitted after their dep anchors
                # so they stream during compute instead of starving the
                # critical xt/w1 sync stream at t=0.
                if mp == 1:
                    dma = nc.gpsimd.dma_start(g_sb[:], g_d[:])
                    _stagger(dma, first_mm.get(1))
                if mp in (2, 5, 8, 11):
                    ciw = (2, 5, 8, 11).index(mp)
                    kstep = (KO2 + 3) // 4
                    k0 = ciw * kstep
                    k1 = min(KO2, k0 + kstep)
                    if k0 < k1:
                        dma = nc.gpsimd.dma_start(
                            w2_sb[:, k0:k1, :], w2_d[:, k0:k1, :]
                        )
                        _stagger(dma, first_mm.get(mp))

            # ---- GEMM2 + gate scale ----
            for ti in range(TT):
                t0 = ti * P
                pt = min(P, C - t0)
                for n0 in (0, 512):
                    psy = ps2.tile([P, 512], f32, tag=f"psy{(2 * ti + n0 // 512) % 2}")
                    for k in range(KO2):
                        nc.tensor.matmul(
                            psy[:pt, :],
                            lhsT=hT[:, k, t0 : t0 + pt],
                            rhs=w2_sb[:, k, n0 : n0 + 512],
                            start=(k == 0),
                            stop=(k == KO2 - 1),
                        )
                    ysb = ev.tile([P, 512], f32, tag="ysb")
                    if has_b2:
                        nc.vector.tensor_tensor(
                            ysb[:pt, :], psy[:pt, :],
                            b2_sb[:, n0 : n0 + 512], ADD
                        )
                        nc.vector.tensor_scalar_mul(
                            ysb[:pt, :], ysb[:pt, :], g_sb[:pt, ti : ti + 1]
                        )
                    else:
                        nc.vector.tensor_scalar_mul(
                            ysb[:pt, :], psy[:pt, :], g_sb[:pt, ti : ti + 1]
                        )
                    nc.sync.dma_start(y_d[t0 : t0 + pt, n0 : n0 + 512], ysb[:pt, :])
    # run_bass_via_pjrt (the axon execute path) takes a prebuilt module and
    # never finalizes it; Bacc defers register allocation to finalize().
    nc.finalize()
    return nc


def _route(x2, Wr):
    """Top-2 router, numpy fp32 (mirrors jax.lax.top_k + softmax)."""
    n = x2.shape[0]
    ar = np.arange(n)
    z = x2 @ Wr  # [N, E] fp32
    idx1 = z.argmax(axis=1)
    v1 = z[ar, idx1]
    z2 = z.copy()
    z2[ar, idx1] = -np.inf
    idx2 = z2.argmax(axis=1)
    v2 = z2[ar, idx2]
    m = np.maximum(v1, v2)
    e1 = np.exp(v1 - m)
    e2 = np.exp(v2 - m)
    s = e1 + e2
    return idx1, idx2, (e1 / s).astype(np.float32), (e2 / s).astype(np.float32)


def kernel(x, Wr, W1, b1, W2, b2):
    x = np.asarray(x, dtype=np.float32)
    Wr = np.asarray(Wr, dtype=np.float32)
    W1 = np.asarray(W1, dtype=np.float32)
    b1 = np.asarray(b1, dtype=np.float32)
    W2 = np.asarray(W2, dtype=np.float32)
    b2 = np.asarray(b2, dtype=np.float32)

    Bb, T, D = x.shape
    E, _, H2 = W1.shape
    H = H2 // 2
    N = Bb * T
    assert E == NCORES

    x2 = x.reshape(N, D)
    idx1, idx2, g1, g2 = _route(x2, Wr)

    tok = np.concatenate([np.arange(N), np.arange(N)])
    exp = np.concatenate([idx1, idx2])
    gat = np.concatenate([g1, g2])

    toks_e = [tok[exp == e] for e in range(E)]
    gats_e = [gat[exp == e] for e in range(E)]
    counts = np.array([len(t) for t in toks_e])
    C = max(512, int(math.ceil(counts.max() / 16) * 16))

    has_b1 = bool(np.any(b1))
    has_b2 = bool(np.any(b2))

    nc = build_moe_expert_nc(D, H, C, has_b1=has_b1, has_b2=has_b2)

    KO1 = D // P
    MP = H // P
    KO2 = H // P
    TT = (C + P - 1) // P
    chunks = _chunks(C)

    in_maps = []
    for e in range(E):
        ce = len(toks_e[e])
        xtf = np.zeros((D, C), dtype=NP_BF16)
        xtf[:, :ce] = x2[toks_e[e]].astype(NP_BF16).T
        # chunk-major pack: per chunk a [P, KO1*cw] contiguous block
        xt_t = np.concatenate(
            [
                xtf[:, c0 : c0 + cw]
                .reshape(KO1, P, cw)
                .transpose(1, 0, 2)
                .reshape(P, KO1 * cw)
                for c0, cw in chunks
            ],
            axis=1,
        )
        xt_t = np.ascontiguousarray(xt_t)

        w1_t = np.ascontiguousarray(
            W1[e].astype(NP_BF16).reshape(KO1, P, 2, MP, P).transpose(3, 1, 2, 0, 4)
        )
        w2_t = np.ascontiguousarray(
            W2[e].astype(NP_BF16).reshape(KO2, P, D).transpose(1, 0, 2)
        )

        g = np.zeros(TT * P, dtype=np.float32)
        g[:ce] = gats_e[e]
        g_t = np.ascontiguousarray(g.reshape(TT, P).T)

        im = {"xt": xt_t, "w1": w1_t, "w2": w2_t, "g": g_t}
        if has_b1:
            im["b1"] = np.ascontiguousarray(
                b1[e].reshape(2, MP, P).transpose(2, 0, 1)
            )
        if has_b2:
            im["b2"] = np.ascontiguousarray(np.broadcast_to(b2[e], (P, D)))
        in_maps.append(im)

    res = run_bass_kernel_spmd(nc, in_maps, list(range(NCORES)))

    out = np.zeros((N, D), dtype=np.float32)
    for e in range(E):
        ce = len(toks_e[e])
        out[toks_e[e]] += res.results[e]["y"][:ce]
    return out.reshape(Bb, T, D)


# revision 4
# speedup vs baseline: 1.1493x; 1.0160x over previous
"""MoE layer (E=8 experts, top-2, swiGLU) on 8 TRN2 NeuronCores.

Strategy: expert parallelism. The router (x @ Wr -> top-2 -> softmax gates)
is tiny (<0.1% of FLOPs) and is computed on host to build the dispatch:
tokens are gathered per expert into a padded capacity-C batch, one expert
per core. Each core runs the expert MLP

    y = (silu(X @ W1a + b1a) * (X @ W1b + b1b)) @ W2 + b2

entirely on device in bf16 (err ~4e-3 << 2e-2 gate; bf16 matmul streams at
the same 1 col/cycle as f32r but halves DMA traffic and enables FWL fast
weight loads). The gate scale and the scatter-add combine happen on host.

Device kernel structure (per core, SPMD — identical program, per-core data):
  Single token block: W1 streamed exactly once; xt/hT/w2 SBUF-resident.
  - xt  [P, KO1*C]   tokens, transposed, chunk-major packed      (resident)
  - w2  [P, KO2, D]  expert W2                                   (resident)
  - hT  [P, MP, C]   swiGLU output, transposed (H on partitions) (resident)
  - W1 streamed from HBM in [P, 2, KO1, 128] column tiles, one per mp
  GEMM1: H1T[h, t] = sum_k W1[k, h] * X[t, k]  (stationary=W1,  moving=xt)
  GEMM2: YT[d, t]  = sum_h W2[h, d] * hT[h, t] (stationary=W2t, moving=hT)
  Both GEMMs stream the token dim as the moving operand in chunks of <=512
  (PSUM bank limit) — tokens never pad to 128-tiles, so total streamed
  columns hit the MAC-count minimum. The C%512 remainder chunk goes FIRST
  in GEMM1 (so the prologue's first matmul group needs only a small xt
  transfer) and LAST in GEMM2 (small final drain). Y leaves transposed
  [d, t]; the host combine undoes it.
  All DMA rides the single sync (HWDGE) queue, whose in-order service is
  exactly the critical path: w1t[0] half-a, xt chunk 0, w1t[0] half-b,
  xt chunks 1.., w1t[1..], w2, then y writes (direct from PSUM).
"""

import math

import numpy as np
import ml_dtypes

import concourse.bacc as bacc
import concourse.bass as bass  # noqa: F401
import concourse.mybir as mybir
import concourse.tile as tile
from concourse.bass_utils import run_bass_kernel_spmd

P = 128
NCORES = 8

f32 = mybir.dt.float32
f32r = mybir.dt.float32r
bf16 = mybir.dt.bfloat16
SIGMOID = mybir.ActivationFunctionType.Sigmoid
ADD = mybir.AluOpType.add

NP_BF16 = ml_dtypes.bfloat16


def _chunks(C):
    """Moving-dim chunks of <=512 covering C, remainder chunk FIRST."""
    rem = C % 512
    out = []
    c0 = 0
    if rem:
        out.append((0, rem))
        c0 = rem
    while c0 < C:
        out.append((c0, 512))
        c0 += 512
    return out


def build_moe_expert_nc(D, H, C, has_b1=False, has_b2=False):
    """Build the SPMD per-expert kernel. D % 128 == 0, H % 128 == 0,
    C % 16 == 0 required."""
    KO1 = D // P       # k tiles of GEMM1 (contraction over D)
    MP = H // P        # hidden tiles (per swiGLU half)
    KO2 = H // P       # k tiles of GEMM2 (contraction over H)
    DP = D // P        # GEMM2 output tiles over D
    chunks = _chunks(C)
    # flat free-dim offset of each chunk in the chunk-major packed xt
    xt_off = [KO1 * c0 for c0, _ in chunks]
    # GEMM2 processes the remainder chunk last
    g2_chunks = chunks[1:] + chunks[:1] if C % 512 else chunks

    nc = bacc.Bacc(None)
    xt_d = nc.declare_dram_parameter("xt", [P, KO1 * C], bf16, isOutput=False)
    w1_d = nc.declare_dram_parameter("w1", [MP, P, 2, KO1, P], bf16, isOutput=False)
    w2_d = nc.declare_dram_parameter("w2", [P, KO2, D], bf16, isOutput=False)
    if has_b1:
        b1_d = nc.declare_dram_parameter("b1", [P, 2, MP], f32, isOutput=False)
    if has_b2:
        b2_d = nc.declare_dram_parameter("b2", [P, DP], f32, isOutput=False)
    y_d = nc.declare_dram_parameter("y", [P, DP, C], f32, isOutput=True)

    with tile.TileContext(nc) as tc:
        with (
            tc.tile_pool(name="sb", bufs=1) as sb,
            tc.tile_pool(name="ps", bufs=1, space="PSUM") as ps,
        ):
            xt_sb = sb.tile([P, KO1 * C], bf16)
            w2_sb = sb.tile([P, KO2, D], bf16)
            hT = sb.tile([P, MP, C], bf16)
            if has_b1:
                b1_sb = sb.tile([P, 2, MP], f32)
                nc.sync.dma_start(b1_sb[:], b1_d[:])
            if has_b2:
                b2_sb = sb.tile([P, DP], f32)
                nc.sync.dma_start(b2_sb[:], b2_d[:])

            # PE warmup: throwaway matmuls on a zeroed scratch tile while the
            # first real loads stream in, so the HAM clock gate is already at
            # 2.4 GHz when real matmuls start. DVE memset (fast) so warmups
            # begin almost immediately after engine boot.
            warm = sb.tile([P, 640], f32r)
            nc.vector.memset(warm[:].bitcast(f32), 0.0)
            for wi in range(8):
                warm_ps = ps.tile([P, 512], f32, tag=f"g1_{wi % 6}",
                                  name=f"warm_ps{wi}")
                nc.tensor.matmul(
                    warm_ps[:],
                    lhsT=warm[:, :128],
                    rhs=warm[:, 128:640],
                    start=True,
                    stop=True,
                )

            # ---- critical-path sync-queue loads (HWDGE service is in-order,
            # so emission order here IS arrival order) ----
            w1t0 = sb.tile([P, 2, KO1, P], bf16, tag="w1t", bufs=3, name="w1t0")
            c00, cw0 = chunks[0]
            nc.sync.dma_start(w1t0[:, 0], w1_d[0, :, 0])          # half a
            nc.sync.dma_start(
                xt_sb[:, xt_off[0] : xt_off[0] + KO1 * cw0],
                xt_d[:, xt_off[0] : xt_off[0] + KO1 * cw0],
            )                                                      # chunk 0
            nc.sync.dma_start(w1t0[:, 1], w1_d[0, :, 1])          # half b
            for ci in range(1, len(chunks)):
                c0, cw = chunks[ci]
                nc.sync.dma_start(
                    xt_sb[:, xt_off[ci] : xt_off[ci] + KO1 * cw],
                    xt_d[:, xt_off[ci] : xt_off[ci] + KO1 * cw],
                )

            def xt_rhs(ci, k, cw):
                off = xt_off[ci] + k * cw
                return xt_sb[:, off : off + cw]

            # ---- GEMM1 + swiGLU ----
            ic = 0
            for mp in range(MP):
                if mp == 0:
                    w1t = w1t0
                else:
                    w1t = sb.tile([P, 2, KO1, P], bf16, tag="w1t", bufs=3)
                    nc.sync.dma_start(w1t[:], w1_d[mp])
                for ci, (c0, cw) in enumerate(chunks):
                    psa = ps.tile([P, 512], f32, tag=f"g1_{(2 * ic) % 6}")
                    psb = ps.tile([P, 512], f32, tag=f"g1_{(2 * ic + 1) % 6}")
                    ic += 1
                    for k in range(KO1):
                        nc.tensor.matmul(
                            psa[:, :cw],
                            lhsT=w1t[:, 0, k, :],
                            rhs=xt_rhs(ci, k, cw),
                            start=(k == 0),
                            stop=(k == KO1 - 1),
                        )
                    for k in range(KO1):
                        nc.tensor.matmul(
                            psb[:, :cw],
                            lhsT=w1t[:, 1, k, :],
                            rhs=xt_rhs(ci, k, cw),
                            start=(k == 0),
                            stop=(k == KO1 - 1),
                        )
                    # silu(a) = a * sigmoid(a); a = psa (+ b1a), b = psb (+ b1b)
                    sg = sb.tile([P, 512], f32, tag="sg", bufs=2)
                    if has_b1:
                        av = sb.tile([P, 512], f32, tag="av", bufs=2)
                        nc.vector.tensor_scalar_add(
                            av[:, :cw], psa[:, :cw], b1_sb[:, 0, mp : mp + 1]
                        )
                        nc.scalar.activation(sg[:, :cw], av[:, :cw], SIGMOID)
                        nc.vector.tensor_mul(sg[:, :cw], sg[:, :cw], av[:, :cw])
                        bs = sb.tile([P, 512], f32, tag="bs", bufs=2)
                        nc.vector.tensor_scalar_add(
                            bs[:, :cw], psb[:, :cw], b1_sb[:, 1, mp : mp + 1]
                        )
                        nc.vector.tensor_mul(
                            hT[:, mp, c0 : c0 + cw], sg[:, :cw], bs[:, :cw]
                        )
                    else:
                        nc.scalar.activation(sg[:, :cw], psa[:, :cw], SIGMOID)
                        nc.vector.tensor_mul(sg[:, :cw], sg[:, :cw], psa[:, :cw])
                        nc.vector.tensor_mul(
                            hT[:, mp, c0 : c0 + cw], sg[:, :cw], psb[:, :cw]
                        )

            # w2 rides the sync queue behind the last w1 tile (arrives ~40 us
            # before GEMM2 needs it); y writes below queue after it.
            nc.sync.dma_start(w2_sb[:], w2_d[:])

            # ---- GEMM2: YT[d, t] — stationary w2 tile, moving hT ----
            iy = 0
            for dp in range(DP):
                for c0, cw in g2_chunks:
                    psy = ps.tile([P, 512], f32, tag=f"psy{iy % 2}")
                    iy += 1
                    for k in range(KO2):
                        nc.tensor.matmul(
                            psy[:, :cw],
                            lhsT=w2_sb[:, k, dp * P : (dp + 1) * P],
                            rhs=hT[:, k, c0 : c0 + cw],
                            start=(k == 0),
                            stop=(k == KO2 - 1),
                        )
                    ysb = sb.tile([P, 512], f32, tag="ysb", bufs=2)
                    if has_b2:
                        nc.vector.tensor_scalar_add(
                            ysb[:, :cw], psy[:, :cw], b2_sb[:, dp : dp + 1]
                        )
                    else:
                        nc.vector.tensor_copy(ysb[:, :cw], psy[:, :cw])
                    nc.sync.dma_start(y_d[:, dp, c0 : c0 + cw], ysb[:, :cw])
    # run_bass_via_pjrt (the axon execute path) takes a prebuilt module and
    # never finalizes it; Bacc defers register allocation to finalize().
    nc.finalize()
    return nc


def _route(x2, Wr):
    """Top-2 router, numpy fp32 (mirrors jax.lax.top_k + softmax)."""
    n = x2.shape[0]
    ar = np.arange(n)
    z = x2 @ Wr  # [N, E] fp32
    idx1 = z.argmax(axis=1)
    v1 = z[ar, idx1]
    z2 = z.copy()
    z2[ar, idx1] = -np.inf
    idx2 = z2.argmax(axis=1)
    v2 = z2[ar, idx2]
    m = np.maximum(v1, v2)
    e1 = np.exp(v1 - m)
    e2 = np.exp(v2 - m)
    s = e1 + e2
    return idx1, idx2, (e1 / s).astype(np.float32), (e2 / s).astype(np.float32)


def kernel(x, Wr, W1, b1, W2, b2):
    x = np.asarray(x, dtype=np.float32)
    Wr = np.asarray(Wr, dtype=np.float32)
    W1 = np.asarray(W1, dtype=np.float32)
    b1 = np.asarray(b1, dtype=np.float32)
    W2 = np.asarray(W2, dtype=np.float32)
    b2 = np.asarray(b2, dtype=np.float32)

    Bb, T, D = x.shape
    E, _, H2 = W1.shape
    H = H2 // 2
    N = Bb * T
    assert E == NCORES

    x2 = x.reshape(N, D)
    idx1, idx2, g1, g2 = _route(x2, Wr)

    tok = np.concatenate([np.arange(N), np.arange(N)])
    exp = np.concatenate([idx1, idx2])
    gat = np.concatenate([g1, g2])

    toks_e = [tok[exp == e] for e in range(E)]
    gats_e = [gat[exp == e] for e in range(E)]
    counts = np.array([len(t) for t in toks_e])
    C = max(512, int(math.ceil(counts.max() / 16) * 16))

    has_b1 = bool(np.any(b1))
    has_b2 = bool(np.any(b2))

    nc = build_moe_expert_nc(D, H, C, has_b1=has_b1, has_b2=has_b2)

    KO1 = D // P
    MP = H // P
    KO2 = H // P
    DP = D // P
    chunks = _chunks(C)

    in_maps = []
    for e in range(E):
        ce = len(toks_e[e])
        xtf = np.zeros((D, C), dtype=NP_BF16)
        xtf[:, :ce] = x2[toks_e[e]].astype(NP_BF16).T
        # chunk-major pack: per chunk a [P, KO1*cw] contiguous block
        xt_t = np.concatenate(
            [
                xtf[:, c0 : c0 + cw]
                .reshape(KO1, P, cw)
                .transpose(1, 0, 2)
                .reshape(P, KO1 * cw)
                for c0, cw in chunks
            ],
            axis=1,
        )
        xt_t = np.ascontiguousarray(xt_t)

        w1_t = np.ascontiguousarray(
            W1[e].astype(NP_BF16).reshape(KO1, P, 2, MP, P).transpose(3, 1, 2, 0, 4)
        )
        w2_t = np.ascontiguousarray(
            W2[e].astype(NP_BF16).reshape(KO2, P, D).transpose(1, 0, 2)
        )

        im = {"xt": xt_t, "w1": w1_t, "w2": w2_t}
        if has_b1:
            im["b1"] = np.ascontiguousarray(
                b1[e].reshape(2, MP, P).transpose(2, 0, 1)
            )
        if has_b2:
            im["b2"] = np.ascontiguousarray(
                b2[e].reshape(DP, P).T
            )
        in_maps.append(im)

    res = run_bass_kernel_spmd(nc, in_maps, list(range(NCORES)))

    out = np.zeros((N, D), dtype=np.float32)
    for e in range(E):
        ce = len(toks_e[e])
        # y is [P, DP, C] = YT[d % 128, d // 128, t]; undo the transpose and
        # apply the gates host-side
        yt = res.results[e]["y"]
        y2 = yt.transpose(2, 1, 0).reshape(-1, D)[:ce]
        out[toks_e[e]] += gats_e[e][:, None] * y2
    return out.reshape(Bb, T, D)
